# revision 14
# baseline (speedup 1.0000x reference)
"""ConvLSTM decoder (2 ConvLSTM layers + top conv) on 8 Trainium2 cores.

Sharding: data-parallel over batch — B=8, one batch element per core,
weights replicated. The T=10 recurrence runs fully on-core.

Layout: images live in SBUF as zero-padded flat row-major strips: each
64-pixel row padded to WP=65 cols (one shared zero col between rows),
64 rows contiguous, 68-col zero margins at both ends. A 3x3 'SAME' conv
is 9 shifted matmuls accumulated in PSUM: tap (dy,dx) reads the strip
shifted by dy*WP+dx.

All gate-conv strips (h1 x2, h0 x2, x-stacked x2) live in ONE fp8 arena
tile so any two taps — even across the i2h/h2h operands of a cell — can
form a DoubleRow fp8 pair: one matmul with K=256 (2 fp8 weights/cell),
rhs a 3-D AP [K, 2, N] whose middle dim strides between the two taps'
strip positions (HW-validated for arbitrary strides). Gate convs become
pure DR pairs: L1 = 9 MMs (4 h2h pairs + h2h/i2h center pair + 4 i2h
pairs), L0 = 7 MMs (x packed K=64->128 by partition-stacking shifted
copies; lone K=64 tap rides the h2h-center pair with upper-64 weights
zeroed). The top conv stays bf16 (fp8 there fails the 2e-2 gate; fp8
gates alone measure ~1.2e-2).
"""

import numpy as np

B, T, C, H, W = 8, 10, 64, 64, 64
CH = 128
NSTEP = T - 1          # 9 recurrent steps
WP = W + 1             # padded row width (one shared zero col per row)
FLAT = H * WP          # 4160
MARG = 68              # >= WP+1 = max |tap offset|
BUFC = MARG + FLAT + MARG
BASE = MARG
HW = H * W             # 4096

# arena regions (fp8): [H1A, H1B, H0A, H0B, XB66, XB2], each BUFC wide
R_H1 = [0 * BUFC, 1 * BUFC]
R_H0 = [2 * BUFC, 3 * BUFC]
R_XA = 4 * BUFC        # x stacked with x shifted -WP in partitions 64:128
R_XB = 5 * BUFC        # x stacked with x shifted -2
AW = 6 * BUFC

# row chunks (r0, r1): 8x7 rows + 2x4 rows; max matmul N = 7*65 = 455 <= 512
CHUNKS = [(i * 7, i * 7 + 7) for i in range(8)] + [(56, 60), (60, 64)]
# weight-sharing groups: matmuls per ldweights = group size; <=4 banks open
CGROUPS = [(0, 4), (4, 8), (8, 10)]

TAPS = [(dy, dx) for dy in (-1, 0, 1) for dx in (-1, 0, 1)]

# std conv DR pairing: (kkA, kkB, offA, strideAB); center tap (1,1) rides
# a cross-operand pair.
STD_PAIRS = [((0, 0), (2, 0), -WP - 1, 2 * WP),
             ((0, 1), (2, 1), -WP, 2 * WP),
             ((0, 2), (2, 2), -WP + 1, 2 * WP),
             ((1, 0), (1, 2), -1, 2)]

# L0 i2h slot packing: slots s0..s3 are K=128 (x stacked with shifted x),
# s4 is the lone K=64 tap (2,1). Slot -> (ky,kx) of (low, high) partitions:
L0SLOT_KK = [((0, 0), (1, 0)), ((0, 1), (1, 1)), ((0, 2), (1, 2)),
             ((2, 0), (2, 2)), ((2, 1), None)]

TOPCT = False         # col-tile the top conv (interp psum-bank check
                       # false-positives on base-partition-64 outputs)
LOOP_N = 0             # >0: wrap body in a hardware repeat loop (timing only)

_CACHE = {}


def _cast_bf16(a):
    import ml_dtypes
    return np.ascontiguousarray(a.astype(ml_dtypes.bfloat16))


def _cast_fp8(a):
    import ml_dtypes
    return np.ascontiguousarray(a.astype(ml_dtypes.float8_e4m3))


def _prep_w_pairs(w):
    """[512, 128, 3, 3] -> [128, 4g*4p*2*128]: the 4 std DR pair lhsTs,
    gate-chunk-major (center tap handled separately)."""
    O, I = w.shape[0], w.shape[1]
    ng = O // CH
    pairs = np.zeros((I, ng, 4, 2, CH), np.float32)
    for g in range(ng):
        sl = slice(g * CH, (g + 1) * CH)
        for p, (ka, kb, _, _) in enumerate(STD_PAIRS):
            pairs[:, g, p, 0, :] = w[sl, :, ka[0], ka[1]].T
            pairs[:, g, p, 1, :] = w[sl, :, kb[0], kb[1]].T
    return _cast_fp8(pairs.reshape(I, -1))


def _prep_center_pair(wh, wx_elem1):
    """Center cross pair: elem0 = wh center tap (K=128), elem1 = wx_elem1
    [K, 4CH] (already K=128 rows, upper rows zeroed if needed).
    -> [128, 4g*2*128]."""
    out = np.zeros((CH, 4, 2, CH), np.float32)
    for g in range(4):
        sl = slice(g * CH, (g + 1) * CH)
        out[:, g, 0, :] = wh[sl, :, 1, 1].T
        out[:, g, 1, :] = wx_elem1[:, sl]
    return _cast_fp8(out.reshape(CH, -1))


def _prep_w0_pairs(w):
    """[512, 64, 3, 3] -> [128, 4g*2p*2*128] slot-stacked DR pairs
    (s0,s1),(s2,s3)."""
    O, I = w.shape[0], w.shape[1]
    ng = O // CH
    pairs = np.zeros((2 * I, ng, 2, 2, CH), np.float32)
    for g in range(ng):
        sl = slice(g * CH, (g + 1) * CH)
        for p in range(2):
            for e in range(2):
                lo, hi = L0SLOT_KK[2 * p + e]
                pairs[:I, g, p, e, :] = w[sl, :, lo[0], lo[1]].T
                if hi is not None:
                    pairs[I:, g, p, e, :] = w[sl, :, hi[0], hi[1]].T
    return _cast_fp8(pairs.reshape(2 * I, -1))


def _prep_w_flat(w):
    # [O, I, 3, 3] -> [I, 9*O]; slice for (tap ti, chunk g): ti*O + g*...
    O, I = w.shape[0], w.shape[1]
    return _cast_bf16(w.transpose(1, 2, 3, 0).reshape(I, 9 * O))


def _build():
    import concourse.bass as bass  # noqa: F401
    import concourse.tile as tile
    from concourse import bacc, mybir
    from concourse.bass_types import AP

    f32 = mybir.dt.float32
    bf16 = mybir.dt.bfloat16
    f8 = mybir.dt.float8e4
    DR = mybir.MatmulPerfMode.DoubleRow
    AF = mybir.ActivationFunctionType

    nc = bacc.Bacc("TRN2", target_bir_lowering=False, debug=False,
                   num_devices=8)

    PW = 4 * 4 * 2 * CH            # 4096: std pairs region cols
    CW = 4 * 2 * CH                # 1024: center-pair region cols
    P0W = 4 * 2 * 2 * CH           # 2048: L0 i2h pairs cols

    xs_d = nc.dram_tensor("xs", [NSTEP, C, HW], f8, kind="ExternalInput")
    h0_d = nc.dram_tensor("h0i", [CH, HW], f8, kind="ExternalInput")
    c0_d = nc.dram_tensor("c0i", [CH, HW], f32, kind="ExternalInput")
    h18_d = nc.dram_tensor("h1i8", [CH, HW], f8, kind="ExternalInput")
    h16_d = nc.dram_tensor("h1i6", [CH, HW], bf16, kind="ExternalInput")
    c1_d = nc.dram_tensor("c1i", [CH, HW], f32, kind="ExternalInput")
    w0_d = nc.dram_tensor("w0", [2 * C, P0W], f8, kind="ExternalInput")
    u0_d = nc.dram_tensor("u0", [CH, PW + CW], f8, kind="ExternalInput")
    w1_d = nc.dram_tensor("w1", [CH, PW], f8, kind="ExternalInput")
    u1_d = nc.dram_tensor("u1", [CH, PW + CW], f8, kind="ExternalInput")
    wt_d = nc.dram_tensor("wt", [CH, 9 * C], bf16, kind="ExternalInput")
    b0_d = nc.dram_tensor("b0", [CH, 4], f32, kind="ExternalInput")
    b1_d = nc.dram_tensor("b1", [CH, 4], f32, kind="ExternalInput")
    bt_d = nc.dram_tensor("bt", [2 * C, 1], f32, kind="ExternalInput")
    out_d = nc.dram_tensor("out", [T, C, HW], f32, kind="ExternalOutput")

    def interior(ap_2d, s0, nrow):
        # rows of 64 interior cols at stride WP starting at flat offset s0
        return ap_2d[:, s0:s0 + nrow * WP].rearrange(
            "p (r c) -> p r c", c=WP)[:, :, 1:1 + W]

    with tile.TileContext(nc) as tc:
        with (
            tc.tile_pool(name="pers", bufs=1) as pers,
            tc.tile_pool(name="ps", bufs=8, space="PSUM") as psp,
            tc.tile_pool(name="gt", bufs=5) as gtp,
            tc.tile_pool(name="osb", bufs=6) as osbp,
        ):
            # --- persistent SBUF residents ---
            w0_t = pers.tile([2 * C, P0W], f8, tag="w0")
            u0_t = pers.tile([CH, PW + CW], f8, tag="u0")
            w1_t = pers.tile([CH, PW], f8, tag="w1")
            u1_t = pers.tile([CH, PW + CW], f8, tag="u1")
            wt_t = pers.tile([CH, 9 * C], bf16, tag="wt")
            b0_t = pers.tile([CH, 4], f32, tag="b0")
            b1_t = pers.tile([CH, 4], f32, tag="b1")
            bt_t = pers.tile([2 * C, 1], f32, tag="bt")
            ar = pers.tile([CH, AW], f8, tag="ar")
            # bf16 copies of the h1 strip for the top conv
            h1q = [pers.tile([CH, BUFC], bf16, tag=f"h1q{i}", name=f"h1q{i}")
                   for i in range(2)]
            c0_t = pers.tile([CH, HW], f32, tag="c0")
            c1_t = pers.tile([CH, HW], f32, tag="c1")

            for t_, d_ in ((w0_t, w0_d), (u0_t, u0_d), (w1_t, w1_d),
                           (u1_t, u1_d), (wt_t, wt_d), (b0_t, b0_d),
                           (b1_t, b1_d), (bt_t, bt_d)):
                nc.sync.dma_start(t_[:], d_.ap())

            # one-time zero fill (margins/padding stay zero forever; the
            # interiors are fully re-written by DMA/compute every iteration)
            nc.vector.memset(ar[:], 0.0)
            for buf in h1q:
                nc.vector.memset(buf[:], 0.0)

            def load_x(t):
                src = xs_d.ap()[t]
                nc.sync.dma_start(interior(ar[:C, :], R_XA + BASE, H), src)
                nc.sync.dma_start(
                    interior(ar[C:2 * C, :], R_XA + BASE - WP, H), src)
                nc.sync.dma_start(interior(ar[:C, :], R_XB + BASE, H), src)
                nc.sync.dma_start(
                    interior(ar[C:2 * C, :], R_XB + BASE - 2, H), src)

            def init_states():
                nc.sync.dma_start(interior(ar, R_H1[0] + BASE, H),
                                  h18_d.ap())
                nc.sync.dma_start(interior(h1q[0], BASE, H), h16_d.ap())
                load_x(0)
                nc.sync.dma_start(interior(ar, R_H0[0] + BASE, H),
                                  h0_d.ap())
                nc.sync.dma_start(c0_t[:], c0_d.ap())
                nc.sync.dma_start(c1_t[:], c1_d.ap())

            def dr_rhs(kk, s, stride, n):
                base = ar[:kk, s:s + n]
                return AP(base.tensor, base.offset,
                          [list(base.ap[0]), [stride, 2], [1, n]])

            def pair3(w_t, o):
                return w_t[:, o:o + 2 * CH].rearrange(
                    "p (two m) -> p two m", two=2)

            # tap emitters: lists of (lhs3d, kk, abs_off_elem0, stride)
            def l0_taps(g, h0base):
                # h2h std pairs first: they read only h0_old, so the x-strip
                # DMA for this step can still be in flight while they run.
                res = []
                for p, (_, _, offa, st) in enumerate(STD_PAIRS):
                    res.append((pair3(u0_t, (g * 4 + p) * 2 * CH), CH,
                                h0base + offa, st))
                # center-combined: h2h center @ h0base, x s4 @ XA+WP
                res.append((pair3(u0_t, PW + g * 2 * CH), CH,
                            h0base, R_XA + WP - h0base))
                # i2h pairs: (s0@XA-WP-1, s1@XA-WP) stride 1;
                #            (s2@XA-WP+1, s3@XB+WP-1) stride BUFC+2WP-2
                res.append((pair3(w0_t, (g * 2) * 2 * CH), 2 * C,
                            R_XA - WP - 1, 1))
                res.append((pair3(w0_t, (g * 2 + 1) * 2 * CH), 2 * C,
                            R_XA - WP + 1, R_XB - R_XA + 2 * WP - 2))
                return res

            def l1_taps(g, h1base, h0base):
                res = []
                # h2h std pairs on h1 strip
                for p, (_, _, offa, st) in enumerate(STD_PAIRS):
                    res.append((pair3(u1_t, (g * 4 + p) * 2 * CH), CH,
                                h1base + offa, st))
                # center pair: h2h center @ h1base, i2h center @ h0base
                res.append((pair3(u1_t, PW + g * 2 * CH), CH,
                            h1base, h0base - h1base))
                # i2h std pairs on h0 (x1) strip
                for p, (_, _, offa, st) in enumerate(STD_PAIRS):
                    res.append((pair3(w1_t, (g * 4 + p) * 2 * CH), CH,
                                h0base + offa, st))
                return res

            def conv_gates(taps_fn, b_t, c_t, houts):
                """One ConvLSTM cell; chunk groups share stationary
                weights. houts: [(tile_or_arena, base_col), ...]."""
                for bi, be in CGROUPS:
                    pair = CHUNKS[bi:be]
                    gtiles = [[None] * 4 for _ in pair]
                    for g in range(4):
                        pss = [psp.tile([CH, (r1 - r0) * WP], f32, tag="ps",
                                        name="ps") for (r0, r1) in pair]
                        taps = taps_fn(g)
                        nt = len(taps)
                        for k, (lhs, kk, off, stride) in enumerate(taps):
                            for j, (r0, r1) in enumerate(pair):
                                s = BASE + r0 * WP + off
                                cw = (r1 - r0) * WP
                                nc.tensor.matmul(
                                    pss[j][:], lhs,
                                    dr_rhs(kk, s, stride, cw),
                                    start=(k == 0), stop=(k == nt - 1),
                                    perf_mode=DR)
                        for j, (r0, r1) in enumerate(pair):
                            nr = r1 - r0
                            gt = gtp.tile([CH, nr * W], f32, tag=f"g{g}",
                                          name=f"g{g}")
                            func = AF.Tanh if g == 2 else AF.Sigmoid
                            nc.scalar.activation(
                                gt[:].rearrange("p (r c) -> p r c", c=W),
                                pss[j][:].rearrange(
                                    "p (r c) -> p r c", c=WP)[:, :, 1:1 + W],
                                func, bias=b_t[:, g:g + 1])
                            gtiles[j][g] = gt
                    for j, (r0, r1) in enumerate(pair):
                        nr = r1 - r0
                        gi, gf, gg, go = gtiles[j]
                        csl = c_t[:, r0 * W:r1 * W]
                        nc.vector.tensor_mul(gg[:], gi[:], gg[:])   # i*g
                        nc.vector.tensor_mul(csl, gf[:], csl)       # f*c
                        nc.vector.tensor_add(csl, csl, gg[:])       # c
                        nc.scalar.activation(gf[:], csl, AF.Tanh)
                        for htile, hb in houts:
                            nc.vector.tensor_mul(
                                interior(htile, hb + BASE + r0 * WP, nr),
                                go[:].rearrange("p (r c) -> p r c", c=W),
                                gf[:].rearrange("p (r c) -> p r c", c=W))

            def conv_top_ct(hin, tout):
                # col-tiled: chunk 2i in PE col-group 0 (psum parts 0:64),
                # chunk 2i+1 in col-group 1 (parts 64:128); same weights per
                # tap, both chains run concurrently on disjoint array halves.
                for pi in range(5):
                    ra, rb = CHUNKS[2 * pi], CHUNKS[2 * pi + 1]
                    nr = ra[1] - ra[0]
                    cw = nr * WP
                    ps = psp.tile([2 * C, cw], f32, tag="ps", name="ps")
                    for ti in range(9):
                        dy, dx = TAPS[ti]
                        lhs = wt_t[:, ti * C:(ti + 1) * C]
                        for half, (r0, _) in ((0, ra), (1, rb)):
                            s = BASE + r0 * WP + dy * WP + dx
                            nc.tensor.matmul(
                                ps[half * C:half * C + C, :],
                                lhs, hin[:, s:s + cw],
                                start=(ti == 0), stop=(ti == 8),
                                skip_group_check=True)
                    ot = osbp.tile([2 * C, nr * W], f32, tag="ot",
                                   name="ot")
                    nc.scalar.activation(
                        ot[:].rearrange("p (r c) -> p r c", c=W),
                        ps[:].rearrange(
                            "p (r c) -> p r c", c=WP)[:, :, 1:1 + W],
                        AF.Identity, bias=bt_t[:, 0:1])
                    nc.gpsimd.dma_start(tout[:, ra[0] * W:ra[1] * W],
                                        ot[:C])
                    nc.gpsimd.dma_start(tout[:, rb[0] * W:rb[1] * W],
                                        ot[C:2 * C])

            def conv_top(hin, tout):
                if TOPCT:
                    conv_top_ct(hin, tout)
                    return
                for bi, be in CGROUPS:
                    pair = CHUNKS[bi:be]
                    pss = [psp.tile([C, (r1 - r0) * WP], f32, tag="ps",
                                    name="ps") for (r0, r1) in pair]
                    for ti in range(9):
                        dy, dx = TAPS[ti]
                        lhs = wt_t[:, ti * C:(ti + 1) * C]
                        for j, (r0, r1) in enumerate(pair):
                            s = BASE + r0 * WP + dy * WP + dx
                            cw = (r1 - r0) * WP
                            nc.tensor.matmul(pss[j][:], lhs,
                                             hin[:, s:s + cw],
                                             start=(ti == 0),
                                             stop=(ti == 8))
                    for j, (r0, r1) in enumerate(pair):
                        nr = r1 - r0
                        ot = osbp.tile([C, nr * W], f32, tag="ot",
                                       name="ot")
                        nc.scalar.activation(
                            ot[:].rearrange("p (r c) -> p r c", c=W),
                            pss[j][:].rearrange(
                                "p (r c) -> p r c", c=WP)[:, :, 1:1 + W],
                            AF.Identity, bias=bt_t[:C, 0:1])
                        nc.gpsimd.dma_start(tout[:, r0 * W:r1 * W],
                                            ot[:])

            def body():
                init_states()
                conv_top(h1q[0][:], out_d.ap()[0])
                for t in range(NSTEP):
                    a, b = t % 2, (t + 1) % 2
                    conv_gates(lambda g: l0_taps(g, R_H0[a]),
                               b0_t, c0_t, [(ar, R_H0[b])])
                    if t + 1 < NSTEP:
                        load_x(t + 1)
                    conv_gates(lambda g: l1_taps(g, R_H1[a], R_H0[b]),
                               b1_t, c1_t,
                               [(ar, R_H1[b]), (h1q[b], 0)])
                    conv_top(h1q[b][:], out_d.ap()[t + 1])

            if LOOP_N > 0:
                with tc.For_i(0, LOOP_N, 1):
                    body()
            else:
                body()

    nc.compile()
    return nc


def _get_nc():
    if "nc" not in _CACHE:
        _CACHE["nc"] = _build()
    return _CACHE["nc"]


def kernel(target, h0, c0, h1, c1,
           wi0, bi0, wh0, bh0,
           wi1, bi1, wh1, bh1,
           wtop, btop):
    from concourse.bass_utils import run_bass_kernel_spmd

    nc = _get_nc()

    target = np.asarray(target, np.float32)
    wi0 = np.asarray(wi0, np.float32)
    wh0 = np.asarray(wh0, np.float32)
    wi1 = np.asarray(wi1, np.float32)
    wh1 = np.asarray(wh1, np.float32)

    # L0 center-pair elem1: lone x tap (2,1), K=64 padded to 128 with zeros
    s4 = np.zeros((CH, 4 * CH), np.float32)
    s4[:C, :] = wi0[:, :, 2, 1].T
    # L1 center-pair elem1: i2h center tap (K=128)
    w1c = wi1[:, :, 1, 1].T

    shared = {
        "w0": _prep_w0_pairs(wi0),
        "u0": np.concatenate([_prep_w_pairs(wh0),
                              _prep_center_pair(wh0, s4)], axis=1),
        "w1": _prep_w_pairs(wi1),
        "u1": np.concatenate([_prep_w_pairs(wh1),
                              _prep_center_pair(wh1, w1c)], axis=1),
        "wt": _prep_w_flat(np.asarray(wtop, np.float32)),
        "b0": np.ascontiguousarray(
            (np.asarray(bi0) + np.asarray(bh0)).astype(np.float32)
            .reshape(4, CH).T),
        "b1": np.ascontiguousarray(
            (np.asarray(bi1) + np.asarray(bh1)).astype(np.float32)
            .reshape(4, CH).T),
        "bt": np.ascontiguousarray(np.tile(
            np.asarray(btop, np.float32), 2).reshape(2 * C, 1)),
    }
    in_maps = []
    for b in range(B):
        m = dict(shared)
        m["xs"] = _cast_fp8(target[b, :NSTEP].reshape(NSTEP, C, HW))
        m["h0i"] = _cast_fp8(np.asarray(h0, np.float32)[b].reshape(CH, HW))
        m["c0i"] = np.ascontiguousarray(
            np.asarray(c0, np.float32)[b].reshape(CH, HW))
        h1b = np.asarray(h1, np.float32)[b].reshape(CH, HW)
        m["h1i8"] = _cast_fp8(h1b)
        m["h1i6"] = _cast_bf16(h1b)
        m["c1i"] = np.ascontiguousarray(
            np.asarray(c1, np.float32)[b].reshape(CH, HW))
        in_maps.append(m)

    res = run_bass_kernel_spmd(nc, in_maps, core_ids=list(range(B)))
    out = np.stack([res.results[b]["out"].reshape(T, C, H, W)
                    for b in range(B)])
    return out
